# revision 1
# baseline (speedup 1.0000x reference)
"""Expert-parallel MoE MLP + residual + LayerNorm on 8 Trainium2 NeuronCores.

Reference computes a dense all-expert MLP then masks: out[t] only depends on
expert e = mask[t].  We route: core d gets expert d's weights plus the tokens
assigned to expert d (gathered on host, zero-padded to a fixed capacity C),
computes gelu(x@w1+b1)@w2+b2, adds the residual, applies LayerNorm, and the
host scatters rows back.  No collectives needed: each token's output lives on
exactly one core.

Per-core layout (feature-major for matmul1, token-major after matmul2):
  matmul1: interT[i, t] = sum_h w1[h, i] * x[t, h]   (lhsT=w1 chunk, rhs=x^T)
  gelu+b1 fused in one ACT op (bias is per-partition in feature-major layout)
  matmul2: y[t, h] = sum_i interT[i, t] * w2[i, h]   (lhsT=interT chunk, rhs=w2)
  LayerNorm in token-major layout (reduction along the free dim).
b2 is folded into the residual operand on the host.
"""

import numpy as np
import ml_dtypes

import concourse.bacc as bacc
import concourse.mybir as mybir
import concourse.tile as tile
from concourse.bass_utils import run_bass_kernel_spmd

E, T, H, I = 8, 8192, 768, 3072
P = 128
HK, IK = H // P, I // P  # 6, 24
EPS = 1e-12
N_CORES = 8

F32 = mybir.dt.float32
BF16 = mybir.dt.bfloat16
AF = mybir.ActivationFunctionType
ALU = mybir.AluOpType


def _build(C: int, act=AF.Gelu, reps: int = 1, n_tok: int | None = None):
    """C: DRAM capacity (multiple of 128). n_tok: tokens actually computed
    (n_tok <= C); the tail beyond n_tok is padding nobody reads back."""
    if n_tok is None:
        n_tok = C
    TCN = C // P  # token chunks per core (DRAM layout)
    blocks = []
    off = 0
    while off < n_tok:
        tb = min(512, n_tok - off)
        blocks.append((off, tb))
        off += tb

    nc = bacc.Bacc(None, target_bir_lowering=False)

    xgt_d = nc.dram_tensor("xgt", [HK, P, C], BF16, kind="ExternalInput")
    xres_d = nc.dram_tensor("xres", [TCN, P, H], F32, kind="ExternalInput")
    w1_d = nc.dram_tensor("w1", [HK, P, I], BF16, kind="ExternalInput")
    b1t_d = nc.dram_tensor("b1t", [P, IK], F32, kind="ExternalInput")
    w2_d = nc.dram_tensor("w2", [IK, P, H], BF16, kind="ExternalInput")
    gb_d = nc.dram_tensor("gb", [P, 2, H], F32, kind="ExternalInput")
    out_d = nc.dram_tensor("out", [TCN, P, H], F32, kind="ExternalOutput")

    with tile.TileContext(nc) as tc:
        with (
            tc.tile_pool(name="res", bufs=1) as rpool,
            tc.tile_pool(name="acts", bufs=2) as apool,
            tc.tile_pool(name="ln", bufs=2) as lnpool,
            tc.tile_pool(name="small", bufs=4) as spool,
            tc.tile_pool(name="psA", bufs=4, space="PSUM") as ppa,
            tc.tile_pool(name="psB", bufs=2, space="PSUM") as ppb,
        ):
            epssb = rpool.tile([P, 1], F32)
            nc.gpsimd.memset(epssb[:], EPS)
            b1sb = rpool.tile([P, IK], F32)
            gbsb = rpool.tile([P, 2, H], F32)
            nc.sync.dma_start(b1sb[:], b1t_d[:])
            nc.sync.dma_start(gbsb[:], gb_d[:])

            for _rep in range(reps):
                # Per-chunk tiles so DMA->compute deps are exact: the first
                # matmul fires as soon as w1[0]/xgt[0] land, not after 13MB.
                w1sb = [rpool.tile([P, I], BF16, tag=f"w1_{k}", name=f"w1sb{k}") for k in range(HK)]
                xgtsb = [rpool.tile([P, C], BF16, tag=f"xgt_{k}", name=f"xgtsb{k}") for k in range(HK)]
                w2sb = [rpool.tile([P, H], BF16, tag=f"w2_{k}", name=f"w2sb{k}") for k in range(IK)]
                xressb = [rpool.tile([P, H], F32, tag=f"xres_{c}", name=f"xressb{c}") for c in range(TCN)]

                for k in range(HK):
                    # halves on separate queues: first matmul waits ~half as long
                    nc.sync.dma_start(w1sb[k][:, : I // 2], w1_d[k][:, : I // 2])
                    nc.sync.dma_start(w1sb[k][:, I // 2 :], w1_d[k][:, I // 2 :])
                    nc.sync.dma_start(xgtsb[k][:], xgt_d[k])

                for bi, (boff, tb) in enumerate(blocks):
                    interT = apool.tile([P, IK, 512], BF16, tag="interT")
                    for m in range(IK):
                        if bi == 0 and m == 10:
                            # w2/xres issued mid-block-0 so they don't steal
                            # HBM bandwidth from the critical w1/xgt path, yet
                            # land before stage B needs them.
                            for k2 in range(IK):
                                nc.sync.dma_start(w2sb[k2][:], w2_d[k2])
                            for c in range(TCN):
                                nc.sync.dma_start(xressb[c][:], xres_d[c])
                        ps = ppa.tile([P, 512], F32, tag="psA")
                        for k in range(HK):
                            nc.tensor.matmul(
                                ps[:, :tb],
                                w1sb[k][:, m * P : (m + 1) * P],
                                xgtsb[k][:, boff : boff + tb],
                                start=(k == 0),
                                stop=(k == HK - 1),
                            )
                        nc.scalar.activation(
                            interT[:, m, :tb], ps[:, :tb], act, bias=b1sb[:, m : m + 1]
                        )

                    for tci in range((tb + P - 1) // P):
                        tcg = boff // P + tci
                        toff = tci * P
                        tw = min(P, tb - toff)
                        psy = ppb.tile([P, H], F32, tag="psB")
                        for n0, nw in ((0, 512), (512, 256)):
                            for k in range(IK):
                                nc.tensor.matmul(
                                    psy[:tw, n0 : n0 + nw],
                                    interT[:, k, toff : toff + tw],
                                    w2sb[k][:, n0 : n0 + nw],
                                    start=(k == 0),
                                    stop=(k == IK - 1),
                                )
                        # LayerNorm over H (free dim). (tensor_tensor_reduce
                        # would fuse the residual add with the row sum, but it
                        # crashes the exec unit on hw — use add + reduce_sum.)
                        x = lnpool.tile([P, H], F32, tag="x")
                        nc.vector.tensor_add(x[:tw], psy[:tw], xressb[tcg][:tw])
                        s1 = spool.tile([P, 1], F32, tag="s1")
                        nc.vector.reduce_sum(s1[:tw], x[:tw], axis=mybir.AxisListType.X)
                        sq = lnpool.tile([P, H], F32, tag="sq")
                        s2 = spool.tile([P, 1], F32, tag="s2")
                        nc.scalar.activation(sq[:tw], x[:tw], AF.Square, accum_out=s2[:tw])
                        mu = spool.tile([P, 1], F32, tag="mu")
                        nc.vector.tensor_scalar_mul(mu[:tw], s1[:tw], 1.0 / H)
                        ex2 = spool.tile([P, 1], F32, tag="ex2")
                        nc.vector.tensor_scalar_mul(ex2[:tw], s2[:tw], 1.0 / H)
                        mu2 = spool.tile([P, 1], F32, tag="mu2")
                        nc.vector.tensor_mul(mu2[:tw], mu[:tw], mu[:tw])
                        var = spool.tile([P, 1], F32, tag="var")
                        nc.vector.tensor_sub(var[:tw], ex2[:tw], mu2[:tw])
                        std = spool.tile([P, 1], F32, tag="std")
                        nc.scalar.activation(std[:tw], var[:tw], AF.Sqrt, bias=epssb[:tw])
                        rs = spool.tile([P, 1], F32, tag="rs")
                        nc.vector.reciprocal(rs[:tw], std[:tw])
                        nmr = spool.tile([P, 1], F32, tag="nmr")
                        nc.vector.tensor_scalar(
                            nmr[:tw], mu[:tw], rs[:tw], -1.0, op0=ALU.mult, op1=ALU.mult
                        )
                        o = lnpool.tile([P, H], F32, tag="o")
                        nc.vector.tensor_scalar(
                            o[:tw], x[:tw], rs[:tw], nmr[:tw], op0=ALU.mult, op1=ALU.add
                        )
                        nc.vector.tensor_mul(o[:tw], o[:tw], gbsb[:tw, 0, :])
                        nc.vector.tensor_add(o[:tw], o[:tw], gbsb[:tw, 1, :])
                        nc.sync.dma_start(out_d[tcg][:tw], o[:tw])

    nc.finalize()
    return nc


_NC_CACHE: dict[tuple, object] = {}


def _get_nc(C: int, n_tok: int, reps: int = 1):
    key = (C, n_tok, reps)
    if key not in _NC_CACHE:
        _NC_CACHE[key] = _build(C, reps=reps, n_tok=n_tok)
    return _NC_CACHE[key]


def _prepare(hidden_states, mask, w1, b1, w2, b2, ln_gamma, ln_beta, reps=1):
    hs = np.asarray(hidden_states, dtype=np.float32)
    mk = np.asarray(mask).reshape(-1).astype(np.int64)
    w1 = np.asarray(w1, dtype=np.float32)
    b1 = np.asarray(b1, dtype=np.float32)
    w2 = np.asarray(w2, dtype=np.float32)
    b2 = np.asarray(b2, dtype=np.float32)
    g = np.asarray(ln_gamma, dtype=np.float32)
    bt = np.asarray(ln_beta, dtype=np.float32)

    idxs = [np.nonzero(mk == e)[0] for e in range(E)]
    max_n = max(len(ix) for ix in idxs)
    C = max(256, -(-max_n // P) * P)  # DRAM capacity: multiple of 128
    n_tok = max(256, max_n)  # tokens actually computed
    nc = _get_nc(C, n_tok, reps)
    TCN = C // P

    gb = np.empty((P, 2, H), dtype=np.float32)
    gb[:, 0, :] = g[None, :]
    gb[:, 1, :] = bt[None, :]

    hs2 = hs.reshape(T, H)
    in_maps = []
    for e in range(E):
        ix = idxs[e]
        xg = np.zeros((C, H), dtype=np.float32)
        xg[: len(ix)] = hs2[ix]
        xgt = np.ascontiguousarray(xg.T).astype(ml_dtypes.bfloat16).reshape(HK, P, C)
        xres = (xg + b2[e][None, :]).reshape(TCN, P, H)
        in_maps.append(
            {
                "xgt": xgt,
                "xres": xres,
                "w1": w1[e].astype(ml_dtypes.bfloat16).reshape(HK, P, I),
                "b1t": np.ascontiguousarray(b1[e].reshape(IK, P).T),
                "w2": w2[e].astype(ml_dtypes.bfloat16).reshape(IK, P, H),
                "gb": gb,
            }
        )

    return nc, in_maps, idxs, C


def _scatter(res, idxs, C):
    out = np.empty((T, H), dtype=np.float32)
    for e in range(E):
        ix = idxs[e]
        out[ix] = res.results[e]["out"].reshape(C, H)[: len(ix)]
    return out.reshape(1, T, H)


def kernel(**inputs):
    nc, in_maps, idxs, C = _prepare(**inputs)
    res = run_bass_kernel_spmd(nc, in_maps, list(range(N_CORES)))
    return _scatter(res, idxs, C)

